# revision 18
# baseline (speedup 1.0000x reference)
"""DopDense forward: relu(x @ (w * mult) + b) on 8 trn2 NeuronCores.

Key algebra: w_new = w * mult (per-column scaling) commutes with the matmul,
so out = relu((x @ w) * mult[None, :] + b).  We compute y^T tiles (units on
partitions, batch on free axis) so the per-column mult/bias become
per-partition scale/bias of a fused Relu eviction (scalar-engine activation
or a 2-op vector tensor_scalar).

mult is computed on device: dd[j] = sum_i |w[i,d_j] - old[i,d_j]| (vector
engine), gating logic in j-space, then a multiplicative scatter to columns
as mult = (1 + L^T lfm1) * (1 + R^T rfm1) -- left/right target columns are
each unique, and the single collision (column 0) is handled exactly by the
product.  L/R are built on device from an iota constant via is_equal.

Sharding: data-parallel over the batch axis (8192 rows/core); w, dop state
replicated.  The big matmul runs in bf16, everything else fp32.  The kernel
is memory-bound (~25 MB/core), so DMA traffic is spread across the sync
HWDGE, scalar HWDGE and gpsimd SWDGE queues with few large DMAs.
"""

import numpy as np
import ml_dtypes

import concourse.bass as bass
import concourse.mybir as mybir
import concourse.tile as tile
from concourse import bacc
from concourse.bass_utils import run_bass_kernel_spmd

F32 = mybir.dt.float32
BF16 = mybir.dt.bfloat16
AF = mybir.ActivationFunctionType
ALU = mybir.AluOpType
BF16_NP = np.dtype(ml_dtypes.bfloat16)

N_CORES = 8
B = 65536
NIN = 512
UNITS = 512
N_DOP = 128
SHARD = B // N_CORES          # 8192 batch rows per core
W = 1024                      # batch window per psum tile (2 PSUM banks)
NWP = SHARD // W              # 8 windows per core
KC = NIN // 128               # 4 contraction chunks
CC = UNITS // 128             # 4 unit chunks
THRESHOLD = 0.0
REF_PERIOD = 2.0

# Static dopaminergic-column index math (mirrors reference.py exactly)
DOP_IDX = np.linspace(1, UNITS - 1, N_DOP, dtype=np.int32)
LEFT_OK = ~np.isin(DOP_IDX - 1, DOP_IDX)
RIGHT_OK = ~np.isin(DOP_IDX + 1, DOP_IDX)
LCOL = (DOP_IDX - 1) % UNITS
RCOL = (DOP_IDX + 1) % UNITS

LOK10 = LEFT_OK.astype(np.float32) * np.float32(10.0 / NIN)
ROK10 = RIGHT_OK.astype(np.float32) * np.float32(10.0 / NIN)

_CACHED_NC = None


def build_nc():
    global _CACHED_NC
    if _CACHED_NC is not None:
        return _CACHED_NC
    nc = bacc.Bacc("TRN2", target_bir_lowering=False, debug=False,
                   num_swdge_queues=2)

    xt = nc.dram_tensor("xt", [NWP, 128, KC * W], BF16, kind="ExternalInput")
    # w chunks packed as [128, (k*CC+c)*128 + m] (bf16, matmul stationary)
    wkb = nc.dram_tensor("wkb", [128, KC * CC * 128], BF16, kind="ExternalInput")
    # all aux inputs packed into one wide tensor (small-row DMAs are slow):
    # [:, 0:18] = per-partition vectors (lok10, rok10, indicator, batch_ctr,
    # b0..b3, lcol%128, rcol%128, Lchunkmask[4], Rchunkmask[4]),
    # [:, 18:146] = iota rows, [:, 146:658] = dop columns of w^T,
    # [:, 658:1170] = dop columns of old^T
    NV = 18
    auxs = nc.dram_tensor("auxs", [128, NV + 128], F32, kind="ExternalInput")
    auxb = nc.dram_tensor("auxb", [128, 2 * NIN], F32, kind="ExternalInput")
    yt = nc.dram_tensor("yt", [NWP, 128, CC * W], F32, kind="ExternalOutput")

    with tile.TileContext(nc) as tc:
        with (
            tc.tile_pool(name="const", bufs=1) as const,
            tc.tile_pool(name="aux", bufs=1) as aux,
            tc.tile_pool(name="xa", bufs=3) as xpool,
            tc.tile_pool(name="ob", bufs=5) as opool,
            tc.tile_pool(name="tmp", bufs=2) as tpool,
        ):
            # ---------- input DMAs: few, large, spread over 3 queues ----------
            # aux-critical inputs lead their queues (they gate mult, which
            # gates every eviction)
            wk_sb = const.tile([128, KC * CC * 128], BF16, tag="wk")
            nc.sync.dma_start(wk_sb[:], wkb[:])
            axs_sb = const.tile([128, NV + 128], F32, tag="axs")
            nc.sync.dma_start(axs_sb[:], auxs[:])
            axb_sb = const.tile([128, 2 * NIN], F32, tag="axb")
            nc.sync.dma_start(axb_sb[:], auxb[:])
            v_sb = axs_sb[:, 0:NV]
            io_sb = axs_sb[:, NV:NV + 128]
            wd_sb = axb_sb[:, 0:NIN]
            od_sb = axb_sb[:, NIN:2 * NIN]

            def wk_tile(k, c):
                i = k * CC + c
                return wk_sb[:, i * 128:(i + 1) * 128]

            # x windows: wp0 split for fast start; then alternate sync/gpsimd.
            xa_tiles = {}

            def load_xa(wp):
                xa = xpool.tile([128, KC * W], BF16, tag="xa")
                if wp == 0:
                    nc.sync.dma_start(xa[:, :2 * W], xt[0][:, :2 * W])
                    nc.scalar.dma_start(xa[:, 2 * W:], xt[0][:, 2 * W:])
                else:
                    nc.gpsimd.dma_start(xa[:], xt[wp])
                xa_tiles[wp] = xa

            for wp in range(3):
                load_xa(wp)

            # scatter masks from iota while waiting on wd/od:
            # Lmod[j, m] = 1 iff LCOL[j] % 128 == m (chunk selection happens
            # via the masked rhs columns in the scatter matmul)
            lmod = const.tile([128, 128], BF16, tag="lmod")
            nc.vector.tensor_scalar(lmod[:], io_sb, v_sb[:, 8:9],
                                    None, op0=ALU.is_equal)
            rmod = const.tile([128, 128], BF16, tag="rmod")
            nc.vector.tensor_scalar(rmod[:], io_sb, v_sb[:, 9:10],
                                    None, op0=ALU.is_equal)

            # ---------- aux compute: dd[j] = sum_i |w[i,d_j] - old[i,d_j]| ----
            dch = aux.tile([128, NIN], F32, tag="dch")
            nc.vector.tensor_tensor(dch[:], wd_sb, od_sb, op=ALU.subtract)
            dd = const.tile([128, 1], F32, tag="dd")
            nc.vector.tensor_reduce(
                dd[:], dch[:], axis=mybir.AxisListType.X, op=ALU.add,
                apply_absolute_value=True,
            )
            # active = (dd > THRESHOLD) & ((batch_ctr - indicator) > REF_PERIOD)
            t1 = const.tile([128, 1], F32, tag="t1")
            nc.vector.tensor_tensor(t1[:], v_sb[:, 3:4], v_sb[:, 2:3],
                                    op=ALU.subtract)
            c2 = const.tile([128, 1], F32, tag="c2")
            nc.vector.tensor_scalar(c2[:], t1[:], REF_PERIOD, None, op0=ALU.is_gt)
            c1 = const.tile([128, 1], F32, tag="c1")
            nc.vector.tensor_scalar(c1[:], dd[:], THRESHOLD, None, op0=ALU.is_gt)
            av = const.tile([128, 1], F32, tag="av")
            nc.vector.tensor_tensor(av[:], c1[:], c2[:], op=ALU.mult)
            da = const.tile([128, 1], F32, tag="da")
            nc.vector.tensor_tensor(da[:], dd[:], av[:], op=ALU.mult)
            lf1 = const.tile([128, 1], F32, tag="lf1")
            nc.vector.tensor_tensor(lf1[:], da[:], v_sb[:, 0:1], op=ALU.mult)
            rf1 = const.tile([128, 1], F32, tag="rf1")
            nc.vector.tensor_tensor(rf1[:], da[:], v_sb[:, 1:2], op=ALU.mult)

            # additive scatters (all 4 chunks in one matmul pair), then
            # mult = (1 + L^T lfm1) * (1 + R^T rfm1).  The two tiny scatter
            # matmuls are emitted mid-way through wp0's groups so the PE can
            # start on the big matmuls before the mult chain resolves.
            lfc = const.tile([128, CC], BF16, tag="lfc")
            nc.vector.tensor_scalar(lfc[:], v_sb[:, 10:10 + CC], lf1[:],
                                    None, op0=ALU.mult)
            rfc = const.tile([128, CC], BF16, tag="rfc")
            nc.vector.tensor_scalar(rfc[:], v_sb[:, 14:14 + CC], rf1[:],
                                    None, op0=ALU.mult)
            multm = const.tile([128, CC], F32, tag="multm")
            mult_sb = [multm[:, cc:cc + 1] for cc in range(CC)]

            def emit_scatter(psaux):
                psl = psaux.tile([128, CC], F32, tag="auxps")
                nc.tensor.matmul(psl[:], lmod[:], lfc[:], start=True, stop=True)
                psr = psaux.tile([128, CC], F32, tag="auxps")
                nc.tensor.matmul(psr[:], rmod[:], rfc[:], start=True, stop=True)
                lsp = const.tile([128, CC], F32, tag="lsp")
                nc.vector.tensor_scalar(lsp[:], psl[:], 1.0, None, op0=ALU.add)
                rsp = const.tile([128, CC], F32, tag="rsp")
                nc.vector.tensor_scalar(rsp[:], psr[:], 1.0, None, op0=ALU.add)
                nc.vector.tensor_tensor(multm[:], lsp[:], rsp[:], op=ALU.mult)

            # ---------- main: y^T = (w^T x^T) scaled+biased+relu ----------
            # Window pairs share each stationary weight across 4 matmuls.
            def evict_act(ps, ob, c):
                nc.scalar.activation(
                    ob[:, c * W:(c + 1) * W], ps[:], AF.Relu,
                    bias=v_sb[:, 4 + c:5 + c], scale=mult_sb[c])

            def evict_dve(ps, ob, c):
                tmp = tpool.tile([128, W], F32, tag="evt")
                nc.vector.tensor_scalar(
                    tmp[:], ps[:], mult_sb[c], v_sb[:, 4 + c:5 + c],
                    op0=ALU.mult, op1=ALU.add)
                nc.vector.tensor_scalar(
                    ob[:, c * W:(c + 1) * W], tmp[:], 0.0, None, op0=ALU.max)

            with (
                tc.tile_pool(name="ps", bufs=3, space="PSUM") as pspool,
                tc.tile_pool(name="psx", bufs=2, space="PSUM") as psaux,
            ):
                ps0_stash = None
                for wp in range(NWP):
                    if wp + 2 < NWP:
                        load_xa(wp + 2)
                    xa = xa_tiles[wp]
                    ob = opool.tile([128, CC * W], F32, tag="ob")
                    for c in range(CC):
                        ps = pspool.tile([128, W], F32, tag="mps")
                        korder = range(KC) if (wp * CC + c) % 2 == 0 \
                            else range(KC - 1, -1, -1)
                        for ki, k in enumerate(korder):
                            for s in range(W // 512):
                                nc.tensor.matmul(
                                    ps[:, s * 512:(s + 1) * 512],
                                    wk_tile(k, c),
                                    xa[:, k * W + s * 512: k * W + (s + 1) * 512],
                                    start=(ki == 0), stop=(ki == KC - 1),
                                )
                        if wp == 0 and c == 0:
                            # defer this eviction until multm's writer is
                            # traced (Tile dependencies follow trace order)
                            ps0_stash = ps
                            continue
                        if wp == 0 and c == 1:
                            emit_scatter(psaux)
                            evict_act(ps0_stash, ob, 0)
                            eng0 = nc.scalar
                            eng0.dma_start(yt[0][:, :W], ob[:, :W])
                        # 20 ACT / 12 DVE eviction split
                        if c < 2 or (c == 2 and wp % 2 == 0):
                            evict_act(ps, ob, c)
                        else:
                            evict_dve(ps, ob, c)
                        # drain each evicted c-chunk immediately; gpsimd
                        # (slow SWDGE drain) only carries mid-kernel pieces
                        if wp in (1, 2) and c < 2:
                            eng = nc.gpsimd
                        elif wp == 0:
                            eng = nc.scalar
                        elif c < 2:
                            eng = nc.scalar if wp % 2 == 0 else nc.sync
                        else:
                            eng = nc.sync if wp % 2 == 0 else nc.scalar
                        eng.dma_start(yt[wp][:, c * W:(c + 1) * W],
                                      ob[:, c * W:(c + 1) * W])

    nc.compile()
    _CACHED_NC = nc
    return nc


LAST_RESULTS = None


def kernel(x, w, b, dop_weights_old, indicator, batch_ctr):
    global LAST_RESULTS
    x = np.asarray(x, dtype=np.float32)
    w = np.ascontiguousarray(np.asarray(w, dtype=np.float32))
    b_arr = np.asarray(b, dtype=np.float32)
    old = np.asarray(dop_weights_old, dtype=np.float32)
    ind = np.asarray(indicator, dtype=np.float32)
    bc_val = float(np.asarray(batch_ctr).item())

    nc = build_nc()

    # replicated (per-core identical) inputs; all reshapes/gathers are pure
    # data marshaling -- every arithmetic op happens on device
    wkb = np.ascontiguousarray(
        w.reshape(KC, 128, CC, 128).transpose(1, 0, 2, 3)
    ).reshape(128, KC * CC * 128).astype(BF16_NP)
    vcols = [LOK10, ROK10, ind.astype(np.float32),
             np.full(128, bc_val, np.float32)]
    vcols += [b_arr[c * 128:(c + 1) * 128] for c in range(CC)]
    vcols += [(LCOL % 128).astype(np.float32), (RCOL % 128).astype(np.float32)]
    vcols += [(LCOL // 128 == cc).astype(np.float32) for cc in range(CC)]
    vcols += [(RCOL // 128 == cc).astype(np.float32) for cc in range(CC)]
    vecs = np.stack(vcols, axis=1).astype(np.float32)
    iot = np.broadcast_to(np.arange(128, dtype=np.float32), (128, 128))
    auxs = np.ascontiguousarray(np.concatenate(
        [vecs, iot], axis=1, dtype=np.float32))
    auxb = np.ascontiguousarray(np.concatenate(
        [w.T[DOP_IDX], old.T[DOP_IDX]], axis=1, dtype=np.float32))

    common = dict(wkb=wkb, auxs=auxs, auxb=auxb)

    xbf = x.astype(BF16_NP)
    in_maps = []
    for i in range(N_CORES):
        xs = xbf[i * SHARD:(i + 1) * SHARD]          # [8192, 512]
        xtc = np.ascontiguousarray(
            xs.reshape(NWP, W, KC, 128).transpose(0, 3, 2, 1)
        ).reshape(NWP, 128, KC * W)
        in_maps.append(dict(common, xt=xtc))

    res = run_bass_kernel_spmd(nc, in_maps, core_ids=list(range(N_CORES)))
    LAST_RESULTS = res

    out = np.empty((B, UNITS), np.float32)
    for i in range(N_CORES):
        ytc = res.results[i]["yt"].reshape(NWP, 128, CC, W)
        out[i * SHARD:(i + 1) * SHARD] = (
            ytc.transpose(0, 3, 2, 1).reshape(SHARD, UNITS))
    return out
